# revision 1
# baseline (speedup 1.0000x reference)
"""Trainium2 Bass kernel for nn_DNN_24464133718540 (embedding_lookup).

Reference computation:
    emb[b,f]  = tables[f, src[b,f]]            # [B, 45, 256] gather
    h         = emb @ W1 + b1                  # [B, 45, 32]
    out[b,f]  = h @ W2 + b2                    # [B, 45, 1]
    result[b] = sum_f out[b,f]                 # [B, 1]

The MLP is linear (no activation), so with w = W1 @ W2 ([256]) and
c = b1 @ W2 + b2 (scalar):
    result[b] = sum_f tables[f, src[b,f]] . w  +  45 * c

Device kernel (SPMD over 8 cores, features sharded 6/6/6/6/6/5/5/5 with
zero-padding to 6 slots):
  phase 1: stream the core's 6 tables from HBM in ~1 MB chunks; fused
           DVE tensor_tensor_reduce computes per-row dot products with w
           -> scores columns [128 v-partitions, 80 chunk-cols] per table.
  phase 2: PE transpose -> PSUM [79,128]; DMA-flatten to a score row
           [1, 10112]; PE K=1 matmul against ones broadcasts the row to
           all 128 partitions (ScalarE evacuates PSUM -> SBUF).
  phase 3: gpsimd ap_gather: 8 blocks of 16 partitions, each block
           gathers 2048 batch indices from its replicated score row.
  phase 4: DMA one row per block -> DRAM out [6, 8, 2048].
Host: sum the 48 partial rows across cores, add 45*c, reshape [B, 1].
"""

import numpy as np

B, F, V, D, H = 16384, 45, 10000, 256, 32
NF = 6                 # feature slots per core (zero-padded)
NCORES = 8
VCH = 80               # score columns per table (9 full chunks x8 + last x8)
VPAD = VCH * 128       # 10240 flattened score-row length (incl. garbage tail)
NBLK = 8               # batch blocks for the gather
BLK = B // NBLK        # 2048 indices per block

# stream layout: chunk c9<9 covers v in [c9*1024,(c9+1)*1024) as [p=128, j=8]
# with v = c9*1024 + p*8 + j; chunk 9 covers [9216,10000) as [p=98, j=8].
# score(v) lands at cols[p, c9*8+j] -> flattened row position col*128 + p.


def _v_to_pos(v):
    """flattened score-row position for vocab index v (vectorized)."""
    c9 = v // 1024
    r = v % 1024
    return (c9 * 8 + (r % 8)) * 128 + r // 8

_COMPILED = {}


def _feature_slots():
    """feature assignment per core: 6,6,6,6,6,5,5,5."""
    counts = [6, 6, 6, 6, 6, 5, 5, 5]
    slots, start = [], 0
    for c in counts:
        slots.append(list(range(start, start + c)))
        start += c
    assert start == F
    return slots


def _build_program():
    import concourse.bacc as bacc
    import concourse.tile as tile
    from concourse import mybir

    f32 = mybir.dt.float32
    bf16 = mybir.dt.bfloat16
    i16 = mybir.dt.int16

    nc = bacc.Bacc("TRN2", target_bir_lowering=False, debug=False,
                   num_devices=NCORES)

    tables_c = nc.dram_tensor("tables_c", [NF, V, D], f32, kind="ExternalInput")
    w_rep_d = nc.dram_tensor("w_rep", [128, D], f32, kind="ExternalInput")
    ident_d = nc.dram_tensor("ident", [128, 128], f32, kind="ExternalInput")
    idx_d = nc.dram_tensor("idx16", [NF, 128, NBLK * BLK // (16 * NBLK)], i16,
                           kind="ExternalInput")  # [NF, 128, 128]
    out_d = nc.dram_tensor("out_part", [NF, NBLK, BLK], f32, kind="ExternalOutput")

    SROW = BLK // 16  # 128 int16 idx entries per partition per feature

    with tile.TileContext(nc) as tc:
        with (
            tc.tile_pool(name="const", bufs=1) as const_pool,
            tc.tile_pool(name="stream", bufs=6) as stream_pool,
            tc.tile_pool(name="prod", bufs=2) as prod_pool,
            tc.tile_pool(name="cols", bufs=3) as cols_pool,
            tc.tile_pool(name="row", bufs=2) as row_pool,
            tc.tile_pool(name="rep", bufs=2) as rep_pool,
            tc.tile_pool(name="gout", bufs=2) as gout_pool,
            tc.tile_pool(name="pst", bufs=2, space="PSUM") as psum_t_pool,
        ):
            # one-time constants
            w_rep = const_pool.tile([128, D], f32, tag="w")
            nc.sync.dma_start(w_rep[:], w_rep_d.ap())
            ident_t = const_pool.tile([128, 128], f32, tag="ident")
            nc.sync.dma_start(ident_t[:], ident_d.ap())
            idx_t = const_pool.tile([128, NF * SROW], i16, tag="idx")
            nc.sync.dma_start(
                idx_t[:].rearrange("p (f s) -> p f s", f=NF),
                idx_d.ap().rearrange("f p s -> p f s"))

            tab_ap = tables_c.ap()  # [NF, V, D]

            # A dma_start whose dependency semaphore is not yet satisfied
            # stalls the *issuing engine's* instruction queue (the wait sits
            # on the doorbell, not the ring descriptor). So phase 2 is split
            # in two stages pipelined at different depths: by the time each
            # doorbell is reached, its wait is already satisfied and the
            # scalar ring keeps streaming table chunks without stalls.
            rep_hold = {}

            def phase2a(f, cols):
                # transpose -> PSUM evac -> flatten row into rep partition 0.
                # Issued one table late: the transpose's input is complete, so
                # the whole chain runs back-to-back with no engine stalls.
                pt = psum_t_pool.tile([VCH, 128], f32, tag="pt")
                nc.tensor.transpose(pt[:], cols[:, :VCH], ident_t[:])
                ptsb = row_pool.tile([VCH, 128], f32, tag="ptsb")
                # PSUM evac on DVE, not ScalarE: ScalarE must stay a pure
                # doorbell queue or a compute op waiting on the previous
                # table's DVE end gates the next table's stream doorbells
                # behind it (zero prefetch depth for the odd chunks).
                nc.vector.tensor_copy(ptsb[:], pt[:])
                rep = rep_pool.tile([128, VPAD], f32, tag="rep")
                nc.scalar.dma_start(
                    rep[0:1, :].rearrange("o (c p) -> o c p", c=VCH), ptsb[:])
                rep_hold[f] = rep

            def phase2b(f):
                # 7 independent 40KB copies p0 -> p{16,32,...,112} (the gather
                # reads only each 16-partition group's base partition), then
                # gather + output. Issued two tables late: the flatten
                # finished a whole table ago, so no doorbell ever waits.
                rep = rep_hold.pop(f)
                repv = rep[:].rearrange("(a g) n -> a g n", g=16)
                for a in range(1, 8):
                    nc.scalar.dma_start(repv[a:a + 1, 0], repv[0:1, 0])

                gout = gout_pool.tile([128, BLK], f32, tag="gout")
                nc.gpsimd.ap_gather(
                    out_ap=gout[:],
                    in_ap=rep[:, :VPAD],
                    idxs_ap=idx_t[:, f * SROW:(f + 1) * SROW],
                    channels=128,
                    num_elems=VPAD,
                    d=1,
                    num_idxs=BLK,
                )
                nc.gpsimd.dma_start(
                    out_d.ap()[f],
                    gout[:].rearrange("(k g) n -> k g n", g=16)[:, 0, :])

            cols_hold = {}
            for f in range(NF):
                # issue the deferred phase-2 work for earlier tables FIRST so
                # every doorbell's wait is satisfied (or within ~2us) when the
                # issuing engine reaches it, keeping the stream rings flowing.
                if f >= 1:
                    phase2a(f - 1, cols_hold.pop(f - 1))
                if f >= 2:
                    phase2b(f - 2)
                cols = cols_pool.tile([128, VCH], f32, tag="cols")
                nc.vector.memset(cols[:], 0.0)
                # ---- phase 1: stream + mul + tree-reduce ----
                # 9 full chunks of 1024 v ([p=128, j=8]), then [p=98, j=8].
                # Chunks alternate between the two HW-DGE rings (sync/scalar)
                # so table streaming is not capped by one ring's bandwidth.
                for c9 in range(10):
                    p = 128 if c9 < 9 else 98
                    st = stream_pool.tile([128, 8 * D], f32, tag="st")
                    src_ap = tab_ap[f, c9 * 1024:c9 * 1024 + p * 8, :].rearrange(
                        "(p j) d -> p j d", p=p)
                    ring = nc.sync if c9 % 2 == 0 else nc.scalar
                    ring.dma_start(st[:p], src_ap)
                    # fp32 mul runs at DVE 1x (fp32 source); the reduction is
                    # split into two bf16 pairwise adds (2x packed mode) plus a
                    # short 1x tensor_reduce over the last 64 elements.
                    prod = prod_pool.tile([128, 8 * D], bf16, tag="prod")
                    nc.vector.tensor_mul(
                        prod[:p].rearrange("p (j d) -> p j d", j=8),
                        st[:p].rearrange("p (j d) -> p j d", j=8),
                        w_rep[:p].unsqueeze(1).broadcast_to([p, 8, D]),
                    )
                    ph = prod_pool.tile([128, 8 * 128], bf16, tag="ph")
                    pv = prod[:p].rearrange("p (j h d) -> p j h d", j=8, h=2)
                    nc.vector.tensor_add(
                        ph[:p].rearrange("p (j d) -> p j d", j=8),
                        pv[:, :, 0], pv[:, :, 1])
                    pq = prod_pool.tile([128, 8 * 64], bf16, tag="pq")
                    phv = ph[:p].rearrange("p (j h d) -> p j h d", j=8, h=2)
                    nc.vector.tensor_add(
                        pq[:p].rearrange("p (j d) -> p j d", j=8),
                        phv[:, :, 0], phv[:, :, 1])
                    nc.vector.tensor_reduce(
                        cols[:p, c9 * 8:(c9 + 1) * 8],
                        pq[:p].rearrange("p (j d) -> p j d", j=8),
                        axis=mybir.AxisListType.X,
                        op=mybir.AluOpType.add,
                    )

                cols_hold[f] = cols
            phase2a(NF - 1, cols_hold.pop(NF - 1))
            phase2b(NF - 2)
            phase2b(NF - 1)

    nc.compile()
    return nc


def _get_program():
    if "nc" not in _COMPILED:
        _COMPILED["nc"] = _build_program()
    return _COMPILED["nc"]


def kernel(src, tables, W1, b1, W2, b2, _trace=False, _trace_cores=None,
           _tmpdir=None):
    from concourse.bass_utils import run_bass_kernel_spmd

    src = np.asarray(src)
    out_dtype = np.float32
    tables = np.asarray(tables, dtype=np.float32)
    W1 = np.asarray(W1, dtype=np.float32)
    b1 = np.asarray(b1, dtype=np.float32)
    W2 = np.asarray(W2, dtype=np.float32)
    b2 = np.asarray(b2, dtype=np.float32)

    w = (W1 @ W2).reshape(D)                      # [256]
    c = float(b1 @ W2[:, 0] + b2[0])              # scalar per feature
    w_rep = np.ascontiguousarray(np.broadcast_to(w[None, :], (128, D)),
                                 dtype=np.float32)
    ident = np.eye(128, dtype=np.float32)

    slots = _feature_slots()
    src_i = np.asarray(src, dtype=np.int64)

    in_maps = []
    for core in range(NCORES):
        feats = slots[core]
        tc_arr = np.zeros((NF, V, D), dtype=np.float32)
        for i, fg in enumerate(feats):
            tc_arr[i] = tables[fg]
        idx16 = np.zeros((NF, 128, BLK // 16), dtype=np.int16)
        for i, fg in enumerate(feats):
            col = _v_to_pos(src_i[:, fg]).astype(np.int16)   # [16384] row positions
            # idx16[i, 16k+p, s] = pos(src[2048k + 16s + p])
            idx16[i] = (col.reshape(NBLK, BLK // 16, 16)
                        .transpose(0, 2, 1)
                        .reshape(128, BLK // 16))
        in_maps.append({
            "tables_c": tc_arr,
            "w_rep": w_rep,
            "ident": ident,
            "idx16": idx16,
        })

    nc = _get_program()
    kw = {}
    if _trace:
        kw = {"trace": True, "trace_cores": _trace_cores or [0],
              "tmpdir": _tmpdir}
    res = run_bass_kernel_spmd(nc, in_maps, core_ids=list(range(NCORES)), **kw)
    _COMPILED["last_results"] = res

    total = np.zeros(B, dtype=np.float64)
    for core in range(NCORES):
        part = res.results[core]["out_part"].reshape(NF, B)
        nf = len(slots[core])
        total += part[:nf].sum(axis=0, dtype=np.float64)
    total += F * c
    return total.astype(out_dtype).reshape(B, 1)



# revision 4
# speedup vs baseline: 1.1770x; 1.1770x over previous
"""Trainium2 Bass kernel for nn_DNN_24464133718540 (embedding_lookup).

Reference computation:
    emb[b,f]  = tables[f, src[b,f]]            # [B, 45, 256] gather
    h         = emb @ W1 + b1                  # [B, 45, 32]
    out[b,f]  = h @ W2 + b2                    # [B, 45, 1]
    result[b] = sum_f out[b,f]                 # [B, 1]

The MLP is linear (no activation), so with w = W1 @ W2 ([256]) and
c = b1 @ W2 + b2 (scalar):
    result[b] = sum_f tables[f, src[b,f]] . w  +  45 * c

Host staging folds w into the tables elementwise (tab*w, cast to bf16)
so the device only row-sums and gathers; the 45 tables are viewed as one
[450000, 256] row space split uniformly across the 8 cores (56250 rows
each = 6 slots x 9375 rows, so every core streams the same 28.8 MB).

Device kernel per slot (SPMD over 8 cores):
  phase 1: stream the slot's 9376 rows from HBM in 1 MB chunks
           ([p=128, j=16 rows, d=256] bf16) alternating the two HW-DGE
           rings (sync/scalar); DVE tree-adds halves (2x packed bf16)
           then tensor_reduce -> scores cols [128, 80] fp32.
  phase 2: PE transpose -> PSUM [80,128]; DVE evac; SWDGE (gpsimd) DMA
           flattens to a score row [1, 10240], log-replicates it to the
           8 gather-group base partitions, ap_gather pulls 16384 scores,
           SWDGE DMA -> DRAM out [slot, 8, 2048].
  All phase-2 DMAs ride the gpsimd ring so their cross-slot dependencies
  never stall the stream doorbells on the sync/scalar rings.
Host: scatter-add the 8x6x16384 gathered scores into [B] via the
reference-order maps, add 45*c.
"""

import numpy as np
import ml_dtypes

B, F, V, D, H = 16384, 45, 10000, 256, 32
NCORES = 8
NSLOT = 6                    # virtual tables per core
RPS = 9375                   # rows per slot (45*10000 / 48)
RPS_PAD = 9376               # +1 zero pad row so the last chunk is j*16
CJ = 16                      # rows per partition per chunk
CROWS = 128 * CJ             # 2048 rows per full chunk
NCHUNK = 5                   # 4 full chunks + 1 partial (p=74)
VCH = NCHUNK * CJ            # 80 score columns per slot
VPAD = VCH * 128             # 10240 flattened score-row length
NBLK = 8                     # gather blocks (Q7 groups)
BLK = 2048                   # gathered values per block
GSLOT = NBLK * BLK           # 16384 gather slots per table-slot
PAD_POS = (16 * 4 + 15) * 128 + 73   # flattened pos of zero pad row 9375

_COMPILED = {}


def _pos_of_local(local):
    """flattened score-row position for local row index (vectorized)."""
    c = local // CROWS
    r = local % CROWS
    return (c * CJ + r % CJ) * 128 + r // CJ


def _build_program():
    import concourse.bacc as bacc
    import concourse.tile as tile
    from concourse import mybir

    f32 = mybir.dt.float32
    bf16 = mybir.dt.bfloat16
    i16 = mybir.dt.int16

    nc = bacc.Bacc("TRN2", target_bir_lowering=False, debug=False,
                   num_devices=NCORES)

    tables_c = nc.dram_tensor("tables_c", [NSLOT, RPS_PAD, D], bf16,
                              kind="ExternalInput")
    ident_d = nc.dram_tensor("ident", [128, 128], f32, kind="ExternalInput")
    idx_d = nc.dram_tensor("idx16", [NSLOT, 128, GSLOT // 128], i16,
                           kind="ExternalInput")  # [NSLOT, 128, 128]
    out_d = nc.dram_tensor("out_part", [NSLOT, NBLK, BLK], f32,
                           kind="ExternalOutput")

    SROW = GSLOT // 128  # 128 int16 idx entries per channel per slot

    with tile.TileContext(nc) as tc:
        with (
            tc.tile_pool(name="const", bufs=1) as const_pool,
            tc.tile_pool(name="stream", bufs=6) as stream_pool,
            tc.tile_pool(name="tree", bufs=2) as tree_pool,
            tc.tile_pool(name="cols", bufs=3) as cols_pool,
            tc.tile_pool(name="row", bufs=2) as row_pool,
            tc.tile_pool(name="rep", bufs=2) as rep_pool,
            tc.tile_pool(name="gout", bufs=2) as gout_pool,
            tc.tile_pool(name="pst", bufs=2, space="PSUM") as psum_t_pool,
        ):
            # one-time constants
            ident_t = const_pool.tile([128, 128], f32, tag="ident")
            nc.sync.dma_start(ident_t[:], ident_d.ap())
            idx_t = const_pool.tile([128, NSLOT * SROW], i16, tag="idx")
            nc.sync.dma_start(
                idx_t[:].rearrange("p (f s) -> p f s", f=NSLOT),
                idx_d.ap().rearrange("f p s -> p f s"))

            tab_ap = tables_c.ap()  # [NSLOT, RPS_PAD, D]

            rep_hold = {}

            def phase2a(f, cols):
                # transpose -> PSUM evac -> flatten row into rep partition 0.
                # Issued one slot late so its inputs are complete by the time
                # each engine reaches the op: no queue-head stalls.
                pt = psum_t_pool.tile([VCH, 128], f32, tag="pt")
                nc.tensor.transpose(pt[:], cols[:, :VCH], ident_t[:])
                ptsb = row_pool.tile([VCH, 128], f32, tag="ptsb")
                # PSUM evac on DVE (in-order right after this slot's reduce).
                nc.vector.tensor_copy(ptsb[:], pt[:])
                rep = rep_pool.tile([128, VPAD], f32, tag="rep")
                nc.scalar.dma_start(
                    rep[0:1, :].rearrange("o (c p) -> o c p", c=VCH), ptsb[:])
                rep_hold[f] = rep

            def phase2b(f):
                # replicate p0 -> p{16,32,...,112} (the gather reads each
                # 16-partition group's base partition), then gather + output.
                rep = rep_hold.pop(f)
                repv = rep[:].rearrange("(a g) n -> a g n", g=16)
                for a in range(1, 8):
                    nc.scalar.dma_start(repv[a:a + 1, 0], repv[0:1, 0])

                gout = gout_pool.tile([128, BLK], f32, tag="gout")
                nc.gpsimd.ap_gather(
                    out_ap=gout[:],
                    in_ap=rep[:, :VPAD],
                    idxs_ap=idx_t[:, f * SROW:(f + 1) * SROW],
                    channels=128,
                    num_elems=VPAD,
                    d=1,
                    num_idxs=BLK,
                )
                nc.gpsimd.dma_start(
                    out_d.ap()[f],
                    gout[:].rearrange("(k g) n -> k g n", g=16)[:, 0, :])

            cols_hold = {}
            for f in range(NSLOT):
                if f >= 1:
                    phase2a(f - 1, cols_hold.pop(f - 1))
                if f >= 2:
                    phase2b(f - 2)
                cols = cols_pool.tile([128, VCH], f32, tag="cols")
                nc.vector.memset(cols[:], 0.0)
                # ---- phase 1: stream + tree-reduce (w pre-folded on host) --
                for c in range(NCHUNK):
                    p = 128 if c < NCHUNK - 1 else (RPS_PAD - 4 * CROWS) // CJ
                    st = stream_pool.tile([128, CJ * D], bf16, tag="st")
                    src_ap = tab_ap[f, c * CROWS:c * CROWS + p * CJ, :].rearrange(
                        "(p j) d -> p j d", p=p)
                    ring = nc.sync if (f * NCHUNK + c) % 2 == 0 else nc.scalar
                    ring.dma_start(st[:p], src_ap)
                    # bf16 pairwise adds run in 2x packed mode; final 32-wide
                    # tensor_reduce emits fp32 score columns.
                    stv = st[:p].rearrange("p (j h d) -> p j h d", j=CJ, h=2)
                    s1 = tree_pool.tile([128, CJ * 128], bf16, tag="s1")
                    nc.vector.tensor_add(
                        s1[:p].rearrange("p (j d) -> p j d", j=CJ),
                        stv[:, :, 0], stv[:, :, 1])
                    s1v = s1[:p].rearrange("p (j h d) -> p j h d", j=CJ, h=2)
                    s2 = tree_pool.tile([128, CJ * 64], bf16, tag="s2")
                    nc.vector.tensor_add(
                        s2[:p].rearrange("p (j d) -> p j d", j=CJ),
                        s1v[:, :, 0], s1v[:, :, 1])
                    s2v = s2[:p].rearrange("p (j h d) -> p j h d", j=CJ, h=2)
                    s3 = tree_pool.tile([128, CJ * 32], bf16, tag="s3")
                    nc.vector.tensor_add(
                        s3[:p].rearrange("p (j d) -> p j d", j=CJ),
                        s2v[:, :, 0], s2v[:, :, 1])
                    nc.vector.tensor_reduce(
                        cols[:p, c * CJ:(c + 1) * CJ],
                        s3[:p].rearrange("p (j d) -> p j d", j=CJ),
                        axis=mybir.AxisListType.X,
                        op=mybir.AluOpType.add,
                    )

                cols_hold[f] = cols
            phase2a(NSLOT - 1, cols_hold.pop(NSLOT - 1))
            phase2b(NSLOT - 2)
            phase2b(NSLOT - 1)

    nc.compile()
    return nc


def _get_program():
    if "nc" not in _COMPILED:
        _COMPILED["nc"] = _build_program()
    return _COMPILED["nc"]


def kernel(src, tables, W1, b1, W2, b2, _trace=False, _trace_cores=None,
           _tmpdir=None):
    from concourse.bass_utils import run_bass_kernel_spmd

    src = np.asarray(src)
    tables = np.asarray(tables, dtype=np.float32)
    W1 = np.asarray(W1, dtype=np.float32)
    b1 = np.asarray(b1, dtype=np.float32)
    W2 = np.asarray(W2, dtype=np.float32)
    b2 = np.asarray(b2, dtype=np.float32)

    w = (W1 @ W2).reshape(D)                      # [256]
    c = float(b1 @ W2[:, 0] + b2[0])              # scalar per feature

    # fold w into the tables and cast to bf16; flatten to one row space
    flat = (tables.reshape(F * V, D) * w[None, :]).astype(ml_dtypes.bfloat16)

    RPC = NSLOT * RPS                             # 56250 rows per core
    src_i = np.asarray(src, dtype=np.int64)
    g = (np.arange(F, dtype=np.int64)[None, :] * V + src_i)   # [B, F]
    core_of = g // RPC
    slot_of = (g % RPC) // RPS
    local = (g % RPC) % RPS
    pos = _pos_of_local(local).astype(np.int16)               # [B, F]
    b_of = np.broadcast_to(np.arange(B, dtype=np.int32)[:, None], (B, F))

    core_flat = core_of.ravel()
    slot_flat = slot_of.ravel()
    pos_flat = pos.ravel()
    b_flat = b_of.ravel()

    in_maps = []
    bmaps = []
    spills = []   # (b_array, g_array) per core for slot overflow (rare)
    for core in range(NCORES):
        tc_arr = np.zeros((NSLOT, RPS_PAD, D), dtype=ml_dtypes.bfloat16)
        rows = flat[core * RPC:(core + 1) * RPC].reshape(NSLOT, RPS, D)
        tc_arr[:, :RPS, :] = rows

        idx16 = np.full((NSLOT, 128, GSLOT // 128), 0, dtype=np.int16)
        bmap = np.zeros((NSLOT, GSLOT), dtype=np.int32)
        sp_b, sp_g = [], []
        msk_core = core_flat == core
        for s in range(NSLOT):
            m = msk_core & (slot_flat == s)
            ps = pos_flat[m]
            bs = b_flat[m]
            n = ps.shape[0]
            if n > GSLOT:
                sp_b.append(bs[GSLOT:])
                sp_g.append(g.ravel()[m.nonzero()[0][GSLOT:]])
                ps, bs, n = ps[:GSLOT], bs[:GSLOT], GSLOT
            full = np.full(GSLOT, PAD_POS, dtype=np.int16)
            full[:n] = ps
            bmap[s, :n] = bs
            # channel c=16k+p gathers ref (2048k + 16s + p) at step s
            idx16[s] = (full.reshape(NBLK, BLK // 16, 16)
                        .transpose(0, 2, 1)
                        .reshape(128, BLK // 16))
        bmaps.append(bmap)
        spills.append((sp_b, sp_g))
        in_maps.append({
            "tables_c": tc_arr,
            "ident": np.eye(128, dtype=np.float32),
            "idx16": idx16,
        })

    nc = _get_program()
    kw = {}
    if _trace:
        kw = {"trace": True, "trace_cores": _trace_cores or [0],
              "tmpdir": _tmpdir}
    res = run_bass_kernel_spmd(nc, in_maps, core_ids=list(range(NCORES)), **kw)
    _COMPILED["last_results"] = res

    total = np.zeros(B, dtype=np.float64)
    for core in range(NCORES):
        vals = res.results[core]["out_part"].reshape(NSLOT, GSLOT)
        np.add.at(total, bmaps[core].ravel(),
                  vals.ravel().astype(np.float64))
        sp_b, sp_g = spills[core]
        for bs, gs in zip(sp_b, sp_g):
            total[bs] += flat[gs].astype(np.float64).sum(axis=1)
    total += F * c
    return total.astype(np.float32).reshape(B, 1)


# revision 8
# speedup vs baseline: 1.2100x; 1.0280x over previous
"""Trainium2 Bass kernel for nn_DNN_24464133718540 (embedding_lookup).

Reference computation:
    emb[b,f]  = tables[f, src[b,f]]            # [B, 45, 256] gather
    h         = emb @ W1 + b1                  # [B, 45, 32]
    out[b,f]  = h @ W2 + b2                    # [B, 45, 1]
    result[b] = sum_f out[b,f]                 # [B, 1]

The MLP is linear (no activation), so with w = W1 @ W2 ([256]) and
c = b1 @ W2 + b2 (scalar):
    result[b] = sum_f tables[f, src[b,f]] . w  +  45 * c

Host staging folds w into the tables elementwise (tab*w, cast to bf16)
so the device only row-sums and gathers; the 45 tables are viewed as one
[450000, 256] row space split uniformly across the 8 cores (56250 rows
each = 6 slots x 9375 rows, so every core streams the same 28.8 MB).

Device kernel per slot (SPMD over 8 cores):
  phase 1: stream the slot's 9376 rows from HBM in 1 MB chunks
           ([p=128, j=16 rows, d=256] bf16) alternating the two HW-DGE
           rings (sync/scalar); DVE tree-adds halves (2x packed bf16)
           then tensor_reduce -> scores cols [128, 80] fp32.
  phase 2: PE transpose -> PSUM [80,128]; DVE evac; SWDGE (gpsimd) DMA
           flattens to a score row [1, 10240], log-replicates it to the
           8 gather-group base partitions, ap_gather pulls 16384 scores,
           SWDGE DMA -> DRAM out [slot, 8, 2048].
  All phase-2 DMAs ride the gpsimd ring so their cross-slot dependencies
  never stall the stream doorbells on the sync/scalar rings.
Host: scatter-add the 8x6x16384 gathered scores into [B] via the
reference-order maps, add 45*c.
"""

import numpy as np
import ml_dtypes

B, F, V, D, H = 16384, 45, 10000, 256, 32
NCORES = 8
NSLOT = 6                    # virtual tables per core
RPS = 9375                   # rows per slot (45*10000 / 48)
RPS_PAD = 9376               # +1 zero pad row so the last chunk is j*16
CJ = 16                      # rows per partition per chunk
CROWS = 128 * CJ             # 2048 rows per full chunk
NCHUNK = 5                   # 4 full chunks + 1 partial (p=74)
VCH = NCHUNK * CJ            # 80 score columns per slot
VPAD = VCH * 128             # 10240 flattened score-row length
NBLK = 8                     # gather blocks (Q7 groups)
BLK = 2048                   # gathered values per block
GSLOT = NBLK * BLK           # 16384 gather slots per table-slot
PAD_POS = (16 * 4 + 15) * 128 + 73   # flattened pos of zero pad row 9375

_COMPILED = {}


def _pos_of_local(local):
    """flattened score-row position for local row index (vectorized)."""
    c = local // CROWS
    r = local % CROWS
    return (c * CJ + r % CJ) * 128 + r // CJ


def _build_program():
    import concourse.bacc as bacc
    import concourse.tile as tile
    from concourse import mybir

    f32 = mybir.dt.float32
    bf16 = mybir.dt.bfloat16
    i16 = mybir.dt.int16

    nc = bacc.Bacc("TRN2", target_bir_lowering=False, debug=False,
                   num_devices=NCORES)

    tables_c = nc.dram_tensor("tables_c", [NSLOT, RPS_PAD, D], bf16,
                              kind="ExternalInput")
    ident_d = nc.dram_tensor("ident", [128, 128], f32, kind="ExternalInput")
    idx_d = nc.dram_tensor("idx16", [NSLOT, 128, GSLOT // 128], i16,
                           kind="ExternalInput")  # [NSLOT, 128, 128]
    out_d = nc.dram_tensor("out_part", [NSLOT, NBLK, BLK], f32,
                           kind="ExternalOutput")

    SROW = GSLOT // 128  # 128 int16 idx entries per channel per slot

    with tile.TileContext(nc) as tc:
        with (
            tc.tile_pool(name="const", bufs=1) as const_pool,
            tc.tile_pool(name="stream", bufs=8) as stream_pool,
            tc.tile_pool(name="tree", bufs=2) as tree_pool,
            tc.tile_pool(name="cols", bufs=4) as cols_pool,
            tc.tile_pool(name="row", bufs=2) as row_pool,
            tc.tile_pool(name="rep", bufs=2) as rep_pool,
            tc.tile_pool(name="gout", bufs=2) as gout_pool,
            tc.tile_pool(name="pst", bufs=2, space="PSUM") as psum_t_pool,
        ):
            # one-time constants
            ident_t = const_pool.tile([128, 128], f32, tag="ident")
            nc.sync.dma_start(ident_t[:], ident_d.ap())
            idx_t = const_pool.tile([128, NSLOT * SROW], i16, tag="idx")
            nc.sync.dma_start(
                idx_t[:].rearrange("p (f s) -> p f s", f=NSLOT),
                idx_d.ap().rearrange("f p s -> p f s"))

            tab_ap = tables_c.ap()  # [NSLOT, RPS_PAD, D]

            rep_hold = {}

            def phase2a(f, cols):
                # transpose -> PSUM evac -> flatten row into rep partition 0.
                # Issued two slots late so its inputs are complete by the time
                # each engine reaches the op: no queue-head stalls.
                pt = psum_t_pool.tile([VCH, 128], f32, tag="pt")
                nc.tensor.transpose(pt[:], cols[:, :VCH], ident_t[:])
                ptsb = row_pool.tile([VCH, 128], f32, tag="ptsb")
                # PSUM evac on DVE (first DVE op of the slot, instant).
                nc.vector.tensor_copy(ptsb[:], pt[:])
                rep = rep_pool.tile([128, VPAD], f32, tag="rep")
                nc.scalar.dma_start(
                    rep[0:1, :].rearrange("o (c p) -> o c p", c=VCH), ptsb[:])
                rep_hold[f] = rep

            def phase2b(f):
                # replicate p0 -> p{16,32,...,112} (the gather reads each
                # 16-partition group's base partition), then gather + output.
                # Issued three slots late; copies split across both HWDGE
                # rings so neither ring eats all the doorbell latency.
                rep = rep_hold.pop(f)
                repv = rep[:].rearrange("(a g) n -> a g n", g=16)
                for a in range(1, 8):
                    ring = nc.sync if a % 2 == 1 else nc.scalar
                    ring.dma_start(repv[a:a + 1, 0], repv[0:1, 0])

                gout = gout_pool.tile([128, BLK], f32, tag="gout")
                nc.gpsimd.ap_gather(
                    out_ap=gout[:],
                    in_ap=rep[:, :VPAD],
                    idxs_ap=idx_t[:, f * SROW:(f + 1) * SROW],
                    channels=128,
                    num_elems=VPAD,
                    d=1,
                    num_idxs=BLK,
                )
                nc.gpsimd.dma_start(
                    out_d.ap()[f],
                    gout[:].rearrange("(k g) n -> k g n", g=16)[:, 0, :])

            cols_hold = {}
            for f in range(NSLOT):
                if f >= 2:
                    phase2a(f - 2, cols_hold.pop(f - 2))
                if f >= 3:
                    phase2b(f - 3)
                cols = cols_pool.tile([128, VCH], f32, tag="cols")
                nc.vector.memset(cols[:], 0.0)
                # ---- phase 1: stream + tree-reduce (w pre-folded on host) --
                for c in range(NCHUNK):
                    p = 128 if c < NCHUNK - 1 else (RPS_PAD - 4 * CROWS) // CJ
                    st = stream_pool.tile([128, CJ * D], bf16, tag="st")
                    src_ap = tab_ap[f, c * CROWS:c * CROWS + p * CJ, :].rearrange(
                        "(p j) d -> p j d", p=p)
                    ring = nc.sync if (f * NCHUNK + c) % 2 == 0 else nc.scalar
                    ring.dma_start(st[:p], src_ap)
                    # bf16 pairwise adds run in 2x packed mode; final 32-wide
                    # tensor_reduce emits fp32 score columns.
                    stv = st[:p].rearrange("p (j h d) -> p j h d", j=CJ, h=2)
                    s1 = tree_pool.tile([128, CJ * 128], bf16, tag="s1")
                    nc.vector.tensor_add(
                        s1[:p].rearrange("p (j d) -> p j d", j=CJ),
                        stv[:, :, 0], stv[:, :, 1])
                    s1v = s1[:p].rearrange("p (j h d) -> p j h d", j=CJ, h=2)
                    s2 = tree_pool.tile([128, CJ * 64], bf16, tag="s2")
                    nc.vector.tensor_add(
                        s2[:p].rearrange("p (j d) -> p j d", j=CJ),
                        s1v[:, :, 0], s1v[:, :, 1])
                    s2v = s2[:p].rearrange("p (j h d) -> p j h d", j=CJ, h=2)
                    s3 = tree_pool.tile([128, CJ * 32], bf16, tag="s3")
                    nc.vector.tensor_add(
                        s3[:p].rearrange("p (j d) -> p j d", j=CJ),
                        s2v[:, :, 0], s2v[:, :, 1])
                    s3v = s3[:p].rearrange("p (j h d) -> p j h d", j=CJ, h=2)
                    s4 = tree_pool.tile([128, CJ * 16], bf16, tag="s4")
                    nc.vector.tensor_add(
                        s4[:p].rearrange("p (j d) -> p j d", j=CJ),
                        s3v[:, :, 0], s3v[:, :, 1])
                    nc.vector.tensor_reduce(
                        cols[:p, c * CJ:(c + 1) * CJ],
                        s4[:p].rearrange("p (j d) -> p j d", j=CJ),
                        axis=mybir.AxisListType.X,
                        op=mybir.AluOpType.add,
                    )

                cols_hold[f] = cols
            phase2a(NSLOT - 2, cols_hold.pop(NSLOT - 2))
            phase2b(NSLOT - 3)
            phase2a(NSLOT - 1, cols_hold.pop(NSLOT - 1))
            phase2b(NSLOT - 2)
            phase2b(NSLOT - 1)

    nc.compile()
    return nc


def _get_program():
    if "nc" not in _COMPILED:
        _COMPILED["nc"] = _build_program()
    return _COMPILED["nc"]


def kernel(src, tables, W1, b1, W2, b2, _trace=False, _trace_cores=None,
           _tmpdir=None):
    from concourse.bass_utils import run_bass_kernel_spmd

    src = np.asarray(src)
    tables = np.asarray(tables, dtype=np.float32)
    W1 = np.asarray(W1, dtype=np.float32)
    b1 = np.asarray(b1, dtype=np.float32)
    W2 = np.asarray(W2, dtype=np.float32)
    b2 = np.asarray(b2, dtype=np.float32)

    w = (W1 @ W2).reshape(D)                      # [256]
    c = float(b1 @ W2[:, 0] + b2[0])              # scalar per feature

    # fold w into the tables and cast to bf16; flatten to one row space
    flat = (tables.reshape(F * V, D) * w[None, :]).astype(ml_dtypes.bfloat16)

    RPC = NSLOT * RPS                             # 56250 rows per core
    src_i = np.asarray(src, dtype=np.int64)
    g = (np.arange(F, dtype=np.int64)[None, :] * V + src_i)   # [B, F]
    core_of = g // RPC
    slot_of = (g % RPC) // RPS
    local = (g % RPC) % RPS
    pos = _pos_of_local(local).astype(np.int16)               # [B, F]
    b_of = np.broadcast_to(np.arange(B, dtype=np.int32)[:, None], (B, F))

    core_flat = core_of.ravel()
    slot_flat = slot_of.ravel()
    pos_flat = pos.ravel()
    b_flat = b_of.ravel()

    in_maps = []
    bmaps = []
    spills = []   # (b_array, g_array) per core for slot overflow (rare)
    for core in range(NCORES):
        tc_arr = np.zeros((NSLOT, RPS_PAD, D), dtype=ml_dtypes.bfloat16)
        rows = flat[core * RPC:(core + 1) * RPC].reshape(NSLOT, RPS, D)
        tc_arr[:, :RPS, :] = rows

        idx16 = np.full((NSLOT, 128, GSLOT // 128), 0, dtype=np.int16)
        bmap = np.zeros((NSLOT, GSLOT), dtype=np.int32)
        sp_b, sp_g = [], []
        msk_core = core_flat == core
        for s in range(NSLOT):
            m = msk_core & (slot_flat == s)
            ps = pos_flat[m]
            bs = b_flat[m]
            n = ps.shape[0]
            if n > GSLOT:
                sp_b.append(bs[GSLOT:])
                sp_g.append(g.ravel()[m.nonzero()[0][GSLOT:]])
                ps, bs, n = ps[:GSLOT], bs[:GSLOT], GSLOT
            full = np.full(GSLOT, PAD_POS, dtype=np.int16)
            full[:n] = ps
            bmap[s, :n] = bs
            # channel c=16k+p gathers ref (2048k + 16s + p) at step s
            idx16[s] = (full.reshape(NBLK, BLK // 16, 16)
                        .transpose(0, 2, 1)
                        .reshape(128, BLK // 16))
        bmaps.append(bmap)
        spills.append((sp_b, sp_g))
        in_maps.append({
            "tables_c": tc_arr,
            "ident": np.eye(128, dtype=np.float32),
            "idx16": idx16,
        })

    nc = _get_program()
    kw = {}
    if _trace:
        kw = {"trace": True, "trace_cores": _trace_cores or [0],
              "tmpdir": _tmpdir}
    res = run_bass_kernel_spmd(nc, in_maps, core_ids=list(range(NCORES)), **kw)
    _COMPILED["last_results"] = res

    total = np.zeros(B, dtype=np.float64)
    for core in range(NCORES):
        vals = res.results[core]["out_part"].reshape(NSLOT, GSLOT)
        np.add.at(total, bmaps[core].ravel(),
                  vals.ravel().astype(np.float64))
        sp_b, sp_g = spills[core]
        for bs, gs in zip(sp_b, sp_g):
            total[bs] += flat[gs].astype(np.float64).sum(axis=1)
    total += F * c
    return total.astype(np.float32).reshape(B, 1)


# revision 10
# speedup vs baseline: 1.3961x; 1.1538x over previous
"""Trainium2 Bass kernel for nn_DNN_24464133718540 (embedding_lookup).

Reference computation:
    emb[b,f]  = tables[f, src[b,f]]            # [B, 45, 256] gather
    h         = emb @ W1 + b1                  # [B, 45, 32]
    out[b,f]  = h @ W2 + b2                    # [B, 45, 1]
    result[b] = sum_f out[b,f]                 # [B, 1]

The MLP is linear (no activation), so with w = W1 @ W2 ([256]) and
c = b1 @ W2 + b2 (scalar):
    result[b] = sum_f tables[f, src[b,f]] . w  +  45 * c

Host staging folds w into the tables elementwise (tab*w, cast to bf16);
the 45 tables are viewed as one [450000, 256] row space split uniformly
across the 8 cores (56250 rows each = 6 slots x 9375 rows). Only the
DISTINCT rows referenced by src need to be touched (~81% of rows), so
the device gathers exactly those via SWDGE dma_gather (HBM row gather,
512 B/row) and row-sums them:

Device kernel per slot (SPMD over 8 cores):
    4 batches x { dma_gather 2048 rows -> [128, 16, 256] bf16 (1 MB);
                  DVE pair-add tree (2x packed) + tensor_reduce
                  -> scores [128, 16] fp32 }
    -> cols [128, 64] -> DMA to DRAM.
Host: scatter-add scores into [B] via the unique-inverse maps, add 45*c.
"""

import numpy as np
import ml_dtypes

B, F, V, D, H = 16384, 45, 10000, 256, 32
NCORES = 8
NSLOT = 6                    # virtual tables per core
RPS = 9375                   # rows per slot (45*10000 / 48)
RPS_PAD = 9376               # +1 zero pad row (pad gather target)
GB = 2048                    # gathered rows per dma_gather batch
NB = 4                       # batches per slot (8192 >= ~7560 distinct)
BJ = GB // 128               # 16 row-blocks per batch
SROW = GB // 16              # 128 idx columns per batch (16-part wrap)
PAD_ROW = RPS                # 9375: the zero row, used for idx padding

_COMPILED = {}


def _build_program():
    import concourse.bacc as bacc
    import concourse.tile as tile
    from concourse import mybir

    f32 = mybir.dt.float32
    bf16 = mybir.dt.bfloat16
    i16 = mybir.dt.int16

    nc = bacc.Bacc("TRN2", target_bir_lowering=False, debug=False,
                   num_devices=NCORES)

    tables_c = nc.dram_tensor("tables_c", [NSLOT, RPS_PAD, D], bf16,
                              kind="ExternalInput")
    idx_d = nc.dram_tensor("gidx", [NSLOT, 128, NB * SROW], i16,
                           kind="ExternalInput")
    out_d = nc.dram_tensor("out_sc", [NSLOT, 128, NB * BJ], f32,
                           kind="ExternalOutput")

    with tile.TileContext(nc) as tc:
        with (
            tc.tile_pool(name="const", bufs=1) as const_pool,
            tc.tile_pool(name="gt", bufs=6) as gt_pool,
            tc.tile_pool(name="tree", bufs=2) as tree_pool,
            tc.tile_pool(name="cols", bufs=2) as cols_pool,
        ):
            idx_t = const_pool.tile([128, NSLOT * NB * SROW], i16, tag="idx")
            nc.sync.dma_start(
                idx_t[:].rearrange("p (f s) -> p f s", f=NSLOT),
                idx_d.ap().rearrange("f p s -> p f s"))

            tab_ap = tables_c.ap()  # [NSLOT, RPS_PAD, D]

            for s in range(NSLOT):
                cols = cols_pool.tile([128, NB * BJ], f32, tag="cols")
                for i in range(NB):
                    gt = gt_pool.tile([128, BJ * D], bf16, tag="gt")
                    nc.gpsimd.dma_gather(
                        gt[:].rearrange("p (b e) -> p b e", b=BJ),
                        tab_ap[s],
                        idx_t[:, (s * NB + i) * SROW:(s * NB + i + 1) * SROW],
                        GB,
                        GB,
                        D,
                        # single-packet mode caps out around 64 descriptors
                        # (~1024 idxs); 2048-row batches need packetization
                        single_packet=False,
                    )
                    # bf16 pairwise adds run in 2x packed mode; the final
                    # 8-wide tensor_reduce emits fp32 score columns.
                    gv = gt[:].rearrange("p (j h d) -> p j h d", j=BJ, h=2)
                    s1 = tree_pool.tile([128, BJ * 128], bf16, tag="s1")
                    nc.vector.tensor_add(
                        s1[:].rearrange("p (j d) -> p j d", j=BJ),
                        gv[:, :, 0], gv[:, :, 1])
                    s1v = s1[:].rearrange("p (j h d) -> p j h d", j=BJ, h=2)
                    s2 = tree_pool.tile([128, BJ * 64], bf16, tag="s2")
                    nc.vector.tensor_add(
                        s2[:].rearrange("p (j d) -> p j d", j=BJ),
                        s1v[:, :, 0], s1v[:, :, 1])
                    s2v = s2[:].rearrange("p (j h d) -> p j h d", j=BJ, h=2)
                    s3 = tree_pool.tile([128, BJ * 32], bf16, tag="s3")
                    nc.vector.tensor_add(
                        s3[:].rearrange("p (j d) -> p j d", j=BJ),
                        s2v[:, :, 0], s2v[:, :, 1])
                    s3v = s3[:].rearrange("p (j h d) -> p j h d", j=BJ, h=2)
                    s4 = tree_pool.tile([128, BJ * 16], bf16, tag="s4")
                    nc.vector.tensor_add(
                        s4[:].rearrange("p (j d) -> p j d", j=BJ),
                        s3v[:, :, 0], s3v[:, :, 1])
                    nc.vector.tensor_reduce(
                        cols[:, i * BJ:(i + 1) * BJ],
                        s4[:].rearrange("p (j d) -> p j d", j=BJ),
                        axis=mybir.AxisListType.X,
                        op=mybir.AluOpType.add,
                    )
                nc.sync.dma_start(out_d.ap()[s], cols[:])

    nc.compile()
    return nc


def _get_program():
    if "nc" not in _COMPILED:
        _COMPILED["nc"] = _build_program()
    return _COMPILED["nc"]


def kernel(src, tables, W1, b1, W2, b2, _trace=False, _trace_cores=None,
           _tmpdir=None):
    from concourse.bass_utils import run_bass_kernel_spmd

    src = np.asarray(src)
    tables = np.asarray(tables, dtype=np.float32)
    W1 = np.asarray(W1, dtype=np.float32)
    b1 = np.asarray(b1, dtype=np.float32)
    W2 = np.asarray(W2, dtype=np.float32)
    b2 = np.asarray(b2, dtype=np.float32)

    w = (W1 @ W2).reshape(D)                      # [256]
    c = float(b1 @ W2[:, 0] + b2[0])              # scalar per feature

    # fold w into the tables and cast to bf16; flatten to one row space
    flat = (tables.reshape(F * V, D) * w[None, :]).astype(ml_dtypes.bfloat16)

    RPC = NSLOT * RPS                             # 56250 rows per core
    GSLOT = NB * GB                               # 8192 gather slots per slot
    src_i = np.asarray(src, dtype=np.int64)
    g = (np.arange(F, dtype=np.int64)[None, :] * V + src_i).ravel()   # [B*F]
    b_of = np.broadcast_to(
        np.arange(B, dtype=np.int32)[:, None], (B, F)).ravel()
    core_of = g // RPC
    slot_of = (g % RPC) // RPS
    local = (g % RPC) % RPS

    in_maps = []
    assembly = []   # per core: list of (b_refs, inverse, n_u, spill_rows)
    for core in range(NCORES):
        tc_arr = np.zeros((NSLOT, RPS_PAD, D), dtype=ml_dtypes.bfloat16)
        rows = flat[core * RPC:(core + 1) * RPC].reshape(NSLOT, RPS, D)
        tc_arr[:, :RPS, :] = rows

        gidx = np.full((NSLOT, 128, NB * SROW), PAD_ROW, dtype=np.int16)
        per_slot = []
        m_core = core_of == core
        for s in range(NSLOT):
            m = m_core & (slot_of == s)
            locs = local[m]
            bs = b_of[m]
            rows_u, inv = np.unique(locs, return_inverse=True)
            n_u = rows_u.shape[0]
            spill = None
            if n_u > GSLOT:
                # rows beyond device capacity: host-summed directly (rare)
                spill = rows_u[GSLOT:]
                keep = inv < GSLOT
                spill_refs = (bs[~keep], inv[~keep] - GSLOT)
                bs, inv = bs[keep], inv[keep]
                rows_u, n_u = rows_u[:GSLOT], GSLOT
                spill = (spill, spill_refs)
            full = np.full(GSLOT, PAD_ROW, dtype=np.int16)
            full[:n_u] = rows_u
            # idx t of batch i -> wrapped [p = t%16, col = i*128 + t//16],
            # replicated across the 8 Q7 core groups of 16 partitions.
            wrapped = (full.reshape(NB, SROW, 16)
                       .transpose(0, 2, 1))           # [NB, 16, SROW]
            gidx[s] = np.tile(
                wrapped.transpose(1, 0, 2).reshape(16, NB * SROW), (8, 1))
            per_slot.append((bs, inv, n_u, spill))
        assembly.append(per_slot)
        in_maps.append({"tables_c": tc_arr, "gidx": gidx})

    nc = _get_program()
    kw = {}
    if _trace:
        kw = {"trace": True, "trace_cores": _trace_cores or [0],
              "tmpdir": _tmpdir}
    res = run_bass_kernel_spmd(nc, in_maps, core_ids=list(range(NCORES)), **kw)
    _COMPILED["last_results"] = res

    total = np.zeros(B, dtype=np.float64)
    for core in range(NCORES):
        out_sc = res.results[core]["out_sc"]          # [NSLOT, 128, NB*BJ]
        for s in range(NSLOT):
            bs, inv, n_u, spill = assembly[core][s]
            # gathered row t = i*2048 + blk*128 + p  ->  out[p, i*16+blk]
            vals_u = (out_sc[s].reshape(128, NB, BJ)
                      .transpose(1, 2, 0).reshape(GSLOT))
            total_add = vals_u[inv].astype(np.float64)
            np.add.at(total, bs, total_add)
            if spill is not None:
                sp_rows, (sp_b, sp_inv) = spill
                base = core * RPC + s * RPS
                sp_scores = flat[base + sp_rows].astype(np.float64).sum(axis=1)
                np.add.at(total, sp_b, sp_scores[sp_inv])
    total += F * c
    return total.astype(np.float32).reshape(B, 1)


# revision 11
# speedup vs baseline: 1.8731x; 1.3417x over previous
"""Trainium2 Bass kernel for nn_DNN_24464133718540 (embedding_lookup).

Reference computation:
    emb[b,f]  = tables[f, src[b,f]]            # [B, 45, 256] gather
    h         = emb @ W1 + b1                  # [B, 45, 32]
    out[b,f]  = h @ W2 + b2                    # [B, 45, 1]
    result[b] = sum_f out[b,f]                 # [B, 1]

The MLP is linear (no activation), so with w = W1 @ W2 ([256]) and
c = b1 @ W2 + b2 (scalar):
    result[b] = sum_f tables[f, src[b,f]] . w  +  45 * c

Host staging folds w into the tables elementwise (tab*w, cast to bf16)
so the device only row-sums and gathers; the 45 tables are viewed as one
[450000, 256] row space split uniformly across the 8 cores (56250 rows
each = 6 slots x 9375 rows, so every core streams the same 28.8 MB).

Device kernel per slot (SPMD over 8 cores):
  phase 1: stream the slot's 9376 rows from HBM in 1 MB chunks
           ([p=128, j=16 rows, d=256] bf16) alternating the two HW-DGE
           rings (sync/scalar); DVE tree-adds halves (2x packed bf16)
           then tensor_reduce -> scores cols [128, 80] fp32.
  phase 2: PE transpose -> PSUM [80,128]; DVE evac; flatten to a score
           row [1, 10240]; replicate to the 8 gather-group base
           partitions; ap_gather pulls the scores of the slot's DISTINCT
           referenced rows only (<= 8192, vs 16384 raw refs - Q7
           ap_gather time is the bottleneck, dedup halves it);
           DMA -> DRAM out [slot, 8, 1024].
  phase2a issues two slots late and phase2b three slots late so every
  phase-2 dependency is ancient by the time an engine reaches the op -
  queue-head waits would otherwise stall the stream doorbells.
Host: scatter-add the gathered distinct-row scores into [B] via the
unique-inverse maps, add 45*c.
"""

import numpy as np
import ml_dtypes

B, F, V, D, H = 16384, 45, 10000, 256, 32
NCORES = 8
NSLOT = 6                    # virtual tables per core
RPS = 9375                   # rows per slot (45*10000 / 48)
RPS_PAD = 9376               # +1 zero pad row so the last chunk is j*16
CJ = 16                      # rows per partition per chunk
CROWS = 128 * CJ             # 2048 rows per full chunk
NCHUNK = 5                   # 4 full chunks + 1 partial (p=74)
VCH = NCHUNK * CJ            # 80 score columns per slot
VPAD = VCH * 128             # 10240 flattened score-row length
NBLK = 8                     # gather blocks (Q7 groups)
BLK = 1024                   # gathered values per block (dedup: 8192/slot)
GSLOT = NBLK * BLK           # 8192 gather slots per table-slot
SROW = BLK // 16             # 64 int16 idx entries per channel per slot
PAD_ROW = RPS                # 9375: the zero pad row

_COMPILED = {}


def _pos_of_local(local):
    """flattened score-row position for local row index (vectorized)."""
    c = local // CROWS
    r = local % CROWS
    return (c * CJ + r % CJ) * 128 + r // CJ


PAD_POS = int(_pos_of_local(np.int64(PAD_ROW)))   # zero score position


def _build_program():
    import concourse.bacc as bacc
    import concourse.tile as tile
    from concourse import mybir

    f32 = mybir.dt.float32
    bf16 = mybir.dt.bfloat16
    i16 = mybir.dt.int16

    nc = bacc.Bacc("TRN2", target_bir_lowering=False, debug=False,
                   num_devices=NCORES)

    tables_c = nc.dram_tensor("tables_c", [NSLOT, RPS_PAD, D], bf16,
                              kind="ExternalInput")
    ident_d = nc.dram_tensor("ident", [128, 128], f32, kind="ExternalInput")
    idx_d = nc.dram_tensor("idx16", [NSLOT, 128, SROW], i16,
                           kind="ExternalInput")
    out_d = nc.dram_tensor("out_part", [NSLOT, NBLK, BLK], f32,
                           kind="ExternalOutput")

    with tile.TileContext(nc) as tc:
        with (
            tc.tile_pool(name="const", bufs=1) as const_pool,
            tc.tile_pool(name="stream", bufs=8) as stream_pool,
            tc.tile_pool(name="tree", bufs=2) as tree_pool,
            tc.tile_pool(name="cols", bufs=4) as cols_pool,
            tc.tile_pool(name="row", bufs=2) as row_pool,
            tc.tile_pool(name="rep", bufs=2) as rep_pool,
            tc.tile_pool(name="gout", bufs=2) as gout_pool,
            tc.tile_pool(name="pst", bufs=2, space="PSUM") as psum_t_pool,
        ):
            # one-time constants
            ident_t = const_pool.tile([128, 128], f32, tag="ident")
            nc.sync.dma_start(ident_t[:], ident_d.ap())
            idx_t = const_pool.tile([128, NSLOT * SROW], i16, tag="idx")
            nc.sync.dma_start(
                idx_t[:].rearrange("p (f s) -> p f s", f=NSLOT),
                idx_d.ap().rearrange("f p s -> p f s"))

            tab_ap = tables_c.ap()  # [NSLOT, RPS_PAD, D]

            rep_hold = {}

            def phase2a(f, cols):
                # transpose -> PSUM evac -> flatten row into rep partition 0.
                pt = psum_t_pool.tile([VCH, 128], f32, tag="pt")
                nc.tensor.transpose(pt[:], cols[:, :VCH], ident_t[:])
                ptsb = row_pool.tile([VCH, 128], f32, tag="ptsb")
                # PSUM evac on DVE (first DVE op of the slot, instant).
                nc.vector.tensor_copy(ptsb[:], pt[:])
                rep = rep_pool.tile([128, VPAD], f32, tag="rep")
                nc.scalar.dma_start(
                    rep[0:1, :].rearrange("o (c p) -> o c p", c=VCH), ptsb[:])
                rep_hold[f] = rep

            def phase2b(f):
                # replicate p0 -> p{16,32,...,112} (the gather reads each
                # 16-partition group's base partition), then gather + output.
                # Copies split across both HWDGE rings.
                rep = rep_hold.pop(f)
                repv = rep[:].rearrange("(a g) n -> a g n", g=16)
                for a in range(1, 8):
                    ring = nc.sync if a % 2 == 1 else nc.scalar
                    ring.dma_start(repv[a:a + 1, 0], repv[0:1, 0])

                gout = gout_pool.tile([128, BLK], f32, tag="gout")
                nc.gpsimd.ap_gather(
                    out_ap=gout[:],
                    in_ap=rep[:, :VPAD],
                    idxs_ap=idx_t[:, f * SROW:(f + 1) * SROW],
                    channels=128,
                    num_elems=VPAD,
                    d=1,
                    num_idxs=BLK,
                )
                nc.gpsimd.dma_start(
                    out_d.ap()[f],
                    gout[:].rearrange("(k g) n -> k g n", g=16)[:, 0, :])

            cols_hold = {}
            for f in range(NSLOT):
                if f >= 2:
                    phase2a(f - 2, cols_hold.pop(f - 2))
                if f >= 3:
                    phase2b(f - 3)
                cols = cols_pool.tile([128, VCH], f32, tag="cols")
                nc.vector.memset(cols[:], 0.0)
                # ---- phase 1: stream + tree-reduce (w pre-folded on host) --
                for c in range(NCHUNK):
                    p = 128 if c < NCHUNK - 1 else (RPS_PAD - 4 * CROWS) // CJ
                    st = stream_pool.tile([128, CJ * D], bf16, tag="st")
                    src_ap = tab_ap[f, c * CROWS:c * CROWS + p * CJ, :].rearrange(
                        "(p j) d -> p j d", p=p)
                    ring = nc.sync if (f * NCHUNK + c) % 2 == 0 else nc.scalar
                    ring.dma_start(st[:p], src_ap)
                    # bf16 pairwise adds run in 2x packed mode; the final
                    # 8-wide tensor_reduce emits fp32 score columns.
                    stv = st[:p].rearrange("p (j h d) -> p j h d", j=CJ, h=2)
                    s1 = tree_pool.tile([128, CJ * 128], bf16, tag="s1")
                    nc.vector.tensor_add(
                        s1[:p].rearrange("p (j d) -> p j d", j=CJ),
                        stv[:, :, 0], stv[:, :, 1])
                    s1v = s1[:p].rearrange("p (j h d) -> p j h d", j=CJ, h=2)
                    s2 = tree_pool.tile([128, CJ * 64], bf16, tag="s2")
                    nc.vector.tensor_add(
                        s2[:p].rearrange("p (j d) -> p j d", j=CJ),
                        s1v[:, :, 0], s1v[:, :, 1])
                    s2v = s2[:p].rearrange("p (j h d) -> p j h d", j=CJ, h=2)
                    s3 = tree_pool.tile([128, CJ * 32], bf16, tag="s3")
                    nc.vector.tensor_add(
                        s3[:p].rearrange("p (j d) -> p j d", j=CJ),
                        s2v[:, :, 0], s2v[:, :, 1])
                    s3v = s3[:p].rearrange("p (j h d) -> p j h d", j=CJ, h=2)
                    s4 = tree_pool.tile([128, CJ * 16], bf16, tag="s4")
                    nc.vector.tensor_add(
                        s4[:p].rearrange("p (j d) -> p j d", j=CJ),
                        s3v[:, :, 0], s3v[:, :, 1])
                    nc.vector.tensor_reduce(
                        cols[:p, c * CJ:(c + 1) * CJ],
                        s4[:p].rearrange("p (j d) -> p j d", j=CJ),
                        axis=mybir.AxisListType.X,
                        op=mybir.AluOpType.add,
                    )

                cols_hold[f] = cols
            phase2a(NSLOT - 2, cols_hold.pop(NSLOT - 2))
            phase2b(NSLOT - 3)
            phase2a(NSLOT - 1, cols_hold.pop(NSLOT - 1))
            phase2b(NSLOT - 2)
            phase2b(NSLOT - 1)

    nc.compile()
    return nc


def _get_program():
    if "nc" not in _COMPILED:
        _COMPILED["nc"] = _build_program()
    return _COMPILED["nc"]


def kernel(src, tables, W1, b1, W2, b2, _trace=False, _trace_cores=None,
           _tmpdir=None):
    from concourse.bass_utils import run_bass_kernel_spmd

    src = np.asarray(src)
    tables = np.asarray(tables, dtype=np.float32)
    W1 = np.asarray(W1, dtype=np.float32)
    b1 = np.asarray(b1, dtype=np.float32)
    W2 = np.asarray(W2, dtype=np.float32)
    b2 = np.asarray(b2, dtype=np.float32)

    w = (W1 @ W2).reshape(D)                      # [256]
    c = float(b1 @ W2[:, 0] + b2[0])              # scalar per feature

    # fold w into the tables and cast to bf16; flatten to one row space
    flat = (tables.reshape(F * V, D) * w[None, :]).astype(ml_dtypes.bfloat16)

    RPC = NSLOT * RPS                             # 56250 rows per core
    src_i = np.asarray(src, dtype=np.int64)
    g = (np.arange(F, dtype=np.int64)[None, :] * V + src_i).ravel()   # [B*F]
    b_of = np.broadcast_to(
        np.arange(B, dtype=np.int32)[:, None], (B, F)).ravel()
    core_of = g // RPC
    slot_of = (g % RPC) // RPS
    local = (g % RPC) % RPS

    in_maps = []
    assembly = []   # per core per slot: (b_refs, inverse, spill)
    for core in range(NCORES):
        tc_arr = np.zeros((NSLOT, RPS_PAD, D), dtype=ml_dtypes.bfloat16)
        rows = flat[core * RPC:(core + 1) * RPC].reshape(NSLOT, RPS, D)
        tc_arr[:, :RPS, :] = rows

        idx16 = np.zeros((NSLOT, 128, SROW), dtype=np.int16)
        per_slot = []
        m_core = core_of == core
        for s in range(NSLOT):
            m = m_core & (slot_of == s)
            locs = local[m]
            bs = b_of[m]
            rows_u, inv = np.unique(locs, return_inverse=True)
            n_u = rows_u.shape[0]
            spill = None
            if n_u > GSLOT:
                # rows beyond device gather capacity: host-summed (rare)
                sp_rows = rows_u[GSLOT:]
                keep = inv < GSLOT
                spill = (sp_rows, bs[~keep], inv[~keep] - GSLOT)
                bs, inv = bs[keep], inv[keep]
                rows_u, n_u = rows_u[:GSLOT], GSLOT
            full = np.full(GSLOT, PAD_POS, dtype=np.int16)
            full[:n_u] = _pos_of_local(rows_u.astype(np.int64)).astype(np.int16)
            # idx16[s, 16k+p, t] = pos of distinct-row number 1024k + 16t + p
            idx16[s] = (full.reshape(NBLK, SROW, 16)
                        .transpose(0, 2, 1)
                        .reshape(128, SROW))
            per_slot.append((bs, inv, spill))
        assembly.append(per_slot)
        in_maps.append({
            "tables_c": tc_arr,
            "ident": np.eye(128, dtype=np.float32),
            "idx16": idx16,
        })

    nc = _get_program()
    kw = {}
    if _trace:
        kw = {"trace": True, "trace_cores": _trace_cores or [0],
              "tmpdir": _tmpdir}
    res = run_bass_kernel_spmd(nc, in_maps, core_ids=list(range(NCORES)), **kw)
    _COMPILED["last_results"] = res

    total = np.zeros(B, dtype=np.float64)
    for core in range(NCORES):
        vals = res.results[core]["out_part"].reshape(NSLOT, GSLOT)
        for s in range(NSLOT):
            bs, inv, spill = assembly[core][s]
            np.add.at(total, bs, vals[s][inv].astype(np.float64))
            if spill is not None:
                sp_rows, sp_b, sp_inv = spill
                base = core * RPC + s * RPS
                sp_scores = flat[base + sp_rows].astype(np.float64).sum(axis=1)
                np.add.at(total, sp_b, sp_scores[sp_inv])
    total += F * c
    return total.astype(np.float32).reshape(B, 1)
